# revision 1
# baseline (speedup 1.0000x reference)
"""Trainium2 Bass kernel for nn_LocationEmbedding (GCN scatter-add + trajectory gather).

Strategy (8 NeuronCores, SPMD):
  - Edges are sharded by target-node (col) range: core k owns nodes
    [k*12500, (k+1)*12500) and receives every edge targeting its range
    (host-side sort/bucketing = sharding layout prep).
  - Launch A: per-core weighted in-degree via segmented reduce, dinv =
    rsqrt(deg), u = dinv * node_feat (bf16). Host concatenates u shards.
  - Launch B: per-core scatter-add z[c] = sum_e w[e] * u[row[e]] via
    indirect-DMA row gathers + one-hot selection matrices contracted on
    the TensorEngine (PSUM accumulation per 128-node block), then
    agg = dinv * ((z + u_local) @ W) + b, relu, and the per-trajectory
    gather of the rows this core owns (packed output; host scatters
    rows into the final [64, 512, 128] tensor).
All arithmetic on device; host does sharding, padding, and index layout.
"""

import os
import numpy as np
import ml_dtypes

import concourse.bass as bass
import concourse.bacc as bacc
import concourse.tile as tile
from concourse import mybir
from concourse.bass_utils import run_bass_kernel_spmd
from concourse.masks import make_identity

BF16 = ml_dtypes.bfloat16
P = 128
N, E, D = 100000, 1600000, 128
NCORES = 8
NS = N // NCORES          # 12500 nodes per core
NB = (NS + P - 1) // P    # 98 blocks per core (last block has 84 rows)
NSPAD = NB * P            # 12544

F32 = mybir.dt.float32
BF = mybir.dt.bfloat16
I32 = mybir.dt.int32

LAST_EXEC_NS = None
LAST_EXEC_PARTS = None


def _build_kernel_a(padw):
    nc = bacc.Bacc("TRN2", target_bir_lowering=False, debug=False)
    wpad = nc.dram_tensor("wpad", [P, NB * padw], F32, kind="ExternalInput")
    nfs = nc.dram_tensor("nfs", [NSPAD, P], F32, kind="ExternalInput")
    u_sh = nc.dram_tensor("u_sh", [NSPAD, P], BF, kind="ExternalOutput")
    dinv_sh = nc.dram_tensor("dinv_sh", [P, NB], F32, kind="ExternalOutput")
    with tile.TileContext(nc) as tc:
        with tc.tile_pool(name="sb", bufs=1) as sb, \
             tc.tile_pool(name="nfp", bufs=4) as nfp:
            w_sb = sb.tile([P, NB * padw], F32)
            nc.sync.dma_start(w_sb[:], wpad[:])
            deg = sb.tile([P, NB], F32)
            nc.vector.tensor_reduce(
                out=deg[:],
                in_=w_sb[:].rearrange("p (b s) -> p b s", s=padw),
                axis=mybir.AxisListType.X,
                op=mybir.AluOpType.add,
            )
            # deg += 1 (self loop), dinv = sqrt(1/deg)
            nc.vector.tensor_scalar_add(deg[:], deg[:], 1.0)
            rec = sb.tile([P, NB], F32)
            nc.vector.reciprocal(rec[:], deg[:])
            dinv = sb.tile([P, NB], F32)
            nc.scalar.activation(dinv[:], rec[:], mybir.ActivationFunctionType.Sqrt)
            nc.sync.dma_start(dinv_sh[:], dinv[:])
            for bi in range(NB):
                t = nfp.tile([P, P], F32, tag="nf")
                nc.sync.dma_start(t[:], nfs[bi * P:(bi + 1) * P, :])
                ub = nfp.tile([P, P], BF, tag="ub")
                nc.vector.tensor_scalar_mul(ub[:], t[:], dinv[:, bi:bi + 1])
                nc.sync.dma_start(u_sh[bi * P:(bi + 1) * P, :], ub[:])
    nc.compile()
    return nc


def _build_kernel_b(cb, j2):
    nc = bacc.Bacc("TRN2", target_bir_lowering=False, debug=False)
    J = NB * cb
    u_full = nc.dram_tensor("u_full", [N, P], BF, kind="ExternalInput")
    u_loc = nc.dram_tensor("u_loc", [NSPAD, P], BF, kind="ExternalInput")
    rows = nc.dram_tensor("rows", [P, J], I32, kind="ExternalInput")
    cl = nc.dram_tensor("cl", [P, J], F32, kind="ExternalInput")
    wch = nc.dram_tensor("wch", [P, J], F32, kind="ExternalInput")
    dinvb = nc.dram_tensor("dinvb", [P, NB], F32, kind="ExternalInput")
    wt = nc.dram_tensor("wt", [P, P], F32, kind="ExternalInput")
    bb = nc.dram_tensor("bb", [P, P], F32, kind="ExternalInput")
    outrows = nc.dram_tensor("outrows", [P, j2], I32, kind="ExternalInput")
    out_packed = nc.dram_tensor("out_packed", [j2 * P, P], F32, kind="ExternalOutput")

    with tile.TileContext(nc) as tc:
        with tc.tile_pool(name="sb", bufs=1) as sb, \
             tc.tile_pool(name="gp", bufs=12) as gp, \
             tc.tile_pool(name="op", bufs=12) as op_, \
             tc.tile_pool(name="blk", bufs=3) as blk, \
             tc.tile_pool(name="ps", bufs=2, space="PSUM") as ps, \
             tc.tile_pool(name="ps2", bufs=2, space="PSUM") as ps2, \
             tc.tile_pool(name="dram", bufs=1, space="DRAM") as dr:
            rows_sb = sb.tile([P, J], I32)
            nc.sync.dma_start(rows_sb[:], rows[:])
            cl_sb = sb.tile([P, J], F32)
            nc.sync.dma_start(cl_sb[:], cl[:])
            w_sb = sb.tile([P, J], F32)
            nc.sync.dma_start(w_sb[:], wch[:])
            dinv_sb = sb.tile([P, NB], F32)
            nc.sync.dma_start(dinv_sb[:], dinvb[:])
            wt_sb = sb.tile([P, P], F32)
            nc.sync.dma_start(wt_sb[:], wt[:])
            bb_sb = sb.tile([P, P], F32)
            nc.sync.dma_start(bb_sb[:], bb[:])
            or_sb = sb.tile([P, j2], I32)
            nc.sync.dma_start(or_sb[:], outrows[:])

            iota_i = sb.tile([P, P], I32)
            nc.gpsimd.iota(iota_i[:], pattern=[[1, P]], channel_multiplier=0)
            iota_f = sb.tile([P, P], F32)
            nc.vector.tensor_copy(iota_f[:], iota_i[:])
            iota_bf = sb.tile([P, P], BF)
            nc.vector.tensor_copy(iota_bf[:], iota_f[:])
            ident = sb.tile([P, P], F32)
            make_identity(nc, ident[:])

            road = dr.tile([NSPAD, P], F32)

            for bi in range(NB):
                h = P if bi < NB - 1 else (NS - (NB - 1) * P)
                zp = ps.tile([P, P], F32, tag="zp")
                for j in range(cb):
                    jj = bi * cb + j
                    ug = gp.tile([P, P], BF, tag="ug")
                    nc.gpsimd.indirect_dma_start(
                        out=ug[:], out_offset=None, in_=u_full[:],
                        in_offset=bass.IndirectOffsetOnAxis(
                            ap=rows_sb[:, jj:jj + 1], axis=0))
                    oh = op_.tile([P, P], BF, tag="oh")
                    nc.vector.tensor_scalar(
                        oh[:], iota_bf[:], cl_sb[:, jj:jj + 1], None,
                        mybir.AluOpType.is_equal)
                    yv = op_.tile([P, P], BF, tag="yv")
                    nc.vector.tensor_scalar(
                        yv[:], ug[:], w_sb[:, jj:jj + 1], None,
                        mybir.AluOpType.mult)
                    nc.tensor.matmul(zp[:], lhsT=oh[:], rhs=yv[:],
                                     start=(j == 0), stop=(j == cb - 1))
                # block tail: s = z + u_local, t = s_T.T @ W, agg/relu
                ul = blk.tile([P, P], BF, tag="ul")
                nc.sync.dma_start(ul[:], u_loc[bi * P:(bi + 1) * P, :])
                uf = blk.tile([P, P], F32, tag="uf")
                nc.vector.tensor_copy(uf[:], ul[:])
                s = blk.tile([P, P], F32, tag="s")
                nc.vector.tensor_tensor(out=s[:], in0=zp[:], in1=uf[:],
                                        op=mybir.AluOpType.add)
                tp = ps2.tile([P, P], F32, tag="tp")
                nc.tensor.transpose(out=tp[:], in_=s[:], identity=ident[:])
                sT = blk.tile([P, P], F32, tag="sT")
                nc.vector.tensor_copy(sT[:], tp[:])
                t2 = ps2.tile([P, P], F32, tag="t2")
                nc.tensor.matmul(t2[:], lhsT=sT[:], rhs=wt_sb[:],
                                 start=True, stop=True)
                r1 = blk.tile([P, P], F32, tag="r1")
                nc.vector.tensor_scalar(
                    r1[:], t2[:], dinv_sb[:, bi:bi + 1], None,
                    mybir.AluOpType.mult)
                nc.vector.tensor_tensor(out=r1[:], in0=r1[:], in1=bb_sb[:],
                                        op=mybir.AluOpType.add)
                nc.vector.tensor_scalar(r1[:], r1[:], 0.0, None,
                                        mybir.AluOpType.max)
                nc.sync.dma_start(road[bi * P:bi * P + h, :], r1[:h, :])

            for j in range(j2):
                og = gp.tile([P, P], F32, tag="og")
                nc.gpsimd.indirect_dma_start(
                    out=og[:], out_offset=None, in_=road[:],
                    in_offset=bass.IndirectOffsetOnAxis(
                        ap=or_sb[:, j:j + 1], axis=0))
                nc.sync.dma_start(out_packed[j * P:(j + 1) * P, :], og[:])
    nc.compile()
    return nc


def kernel(**inputs):
    traj = np.asarray(inputs["traj_seqs"])[..., 0].astype(np.int64)
    seq_len = np.asarray(inputs["seq_len"]).astype(np.int64)
    nf = np.ascontiguousarray(np.asarray(inputs["node_feat"], dtype=np.float32))
    ei = np.asarray(inputs["edge_index"]).astype(np.int64)
    ef = np.asarray(inputs["edge_feat"], dtype=np.float32)
    W = np.ascontiguousarray(np.asarray(inputs["W"], dtype=np.float32))
    b = np.asarray(inputs["b"], dtype=np.float32)

    row, col = ei[0], ei[1]
    owner = col // NS

    # ---------- host layout prep (sharding) ----------
    core_data = []
    padw_g, cb_g = 1, 1
    for k in range(NCORES):
        m = owner == k
        ck = (col[m] - k * NS).astype(np.int64)
        rk = row[m].astype(np.int64)
        wk = ef[m]
        srt = np.argsort(ck, kind="stable")
        cs, rs, ws = ck[srt], rk[srt], wk[srt]
        cnts = np.bincount(cs, minlength=NS)
        padw_g = max(padw_g, int(cnts.max()))
        bcnt = np.bincount(cs // P, minlength=NB)
        cb_g = max(cb_g, int(np.ceil(bcnt.max() / P)))
        core_data.append((cs, rs, ws, cnts))

    flat = traj.reshape(-1)
    posmask = (np.arange(512)[None, :] < seq_len[:, None]).reshape(-1)
    oo = flat // NS
    sels = [np.where((oo == k) & posmask)[0] for k in range(NCORES)]
    j2_g = max(1, int(np.ceil(max(len(s) for s in sels) / P)))

    # ---------- launch A ----------
    in_maps_a = []
    for k in range(NCORES):
        cs, rs, ws, cnts = core_data[k]
        starts = np.zeros(NS, np.int64)
        np.cumsum(cnts[:-1], out=starts[1:])
        posin = np.arange(len(cs)) - starts[cs]
        arr = np.zeros((NSPAD, padw_g), np.float32)
        arr[cs, posin] = ws
        wpad = np.ascontiguousarray(
            arr.reshape(NB, P, padw_g).transpose(1, 0, 2).reshape(P, NB * padw_g))
        nfs = np.zeros((NSPAD, P), np.float32)
        nfs[:NS] = nf[k * NS:(k + 1) * NS]
        in_maps_a.append({"wpad": wpad, "nfs": nfs})

    trace = bool(os.environ.get("KERNEL_TRACE"))
    nca = _build_kernel_a(padw_g)
    ra = run_bass_kernel_spmd(nca, in_maps_a, core_ids=list(range(NCORES)),
                              trace=trace)
    u_full = np.concatenate(
        [ra.results[k]["u_sh"][:NS] for k in range(NCORES)], axis=0)
    u_full = np.ascontiguousarray(u_full)  # [100000, 128] bf16

    # ---------- launch B ----------
    in_maps_b = []
    J = NB * cb_g
    for k in range(NCORES):
        cs, rs, ws, cnts = core_data[k]
        rows_a = np.zeros((P, J), np.int32)
        cl_a = np.zeros((P, J), np.float32)
        w_a = np.zeros((P, J), np.float32)
        bstart = np.searchsorted(cs, np.arange(0, NS + P, P))
        for bi in range(NB):
            lo, hi = int(bstart[bi]), int(bstart[bi + 1])
            n = hi - lo
            rblk = np.zeros(cb_g * P, np.int32)
            clblk = np.zeros(cb_g * P, np.float32)
            wblk = np.zeros(cb_g * P, np.float32)
            rblk[:n] = rs[lo:hi]
            clblk[:n] = cs[lo:hi] - bi * P
            wblk[:n] = ws[lo:hi]
            sl = slice(bi * cb_g, (bi + 1) * cb_g)
            rows_a[:, sl] = rblk.reshape(cb_g, P).T
            cl_a[:, sl] = clblk.reshape(cb_g, P).T
            w_a[:, sl] = wblk.reshape(cb_g, P).T
        orows = np.zeros(j2_g * P, np.int32)
        lv = (flat[sels[k]] - k * NS).astype(np.int32)
        orows[:len(lv)] = lv
        u_loc = np.zeros((NSPAD, P), BF16)
        u_loc[:NS] = u_full[k * NS:(k + 1) * NS]
        in_maps_b.append({
            "u_full": u_full, "u_loc": u_loc, "rows": rows_a, "cl": cl_a,
            "wch": w_a, "dinvb": ra.results[k]["dinv_sh"], "wt": W,
            "bb": np.ascontiguousarray(np.broadcast_to(b, (P, P))).astype(np.float32),
            "outrows": orows.reshape(j2_g, P).T.copy(),
        })

    ncb = _build_kernel_b(cb_g, j2_g)
    rb = run_bass_kernel_spmd(ncb, in_maps_b, core_ids=list(range(NCORES)),
                              trace=trace)
    global LAST_EXEC_NS, LAST_EXEC_PARTS
    LAST_EXEC_PARTS = (ra.exec_time_ns, rb.exec_time_ns)
    if ra.exec_time_ns and rb.exec_time_ns:
        LAST_EXEC_NS = ra.exec_time_ns + rb.exec_time_ns

    # ---------- host assembly ----------
    out = np.zeros((64 * 512, D), np.float32)
    for k in range(NCORES):
        if len(sels[k]):
            out[sels[k]] = rb.results[k]["out_packed"][:len(sels[k])]
    return out.reshape(64, 512, D)



# revision 12
# speedup vs baseline: 1.4427x; 1.4427x over previous
"""Trainium2 Bass kernel for nn_LocationEmbedding (GCN scatter-add + trajectory gather).

Strategy (8 NeuronCores, SPMD), v3:
  - Nodes are assigned to 784 bins of 128 slots (serpentine by in-degree)
    so every bin has a near-equal edge load; core k owns 98 bins (blocks).
  - u_full rows live in 4 segments (<=32768 rows each) so dma_gather's
    int16 indices can address them; a greedy host pass picks each node's
    segment so every (bin, segment) cell fits its chunk quota [5,4,4,4]
    (cb = 17 chunks of 128 edge slots per block).
  - Launch A: per-core weighted in-degree deg[p,b] via one segmented
    reduce over a host-packed weight table, dinv = sqrt(1/deg),
    u = dinv * node_feat in bf16 (grouped ops).
  - Launch B: per (7-block group, segment) ONE dma_gather pulls all the
    group's source rows (SWDGE prep cost amortized ~1us/call); batched
    DVE compare builds the one-hot selection matrices and an in-place
    multiply folds in edge weights; 17 matmuls/block accumulate z^T in
    PSUM (plus an identity matmul seeding the self-loop term); z^T @ W,
    Relu(t * dinv) on the Scalar engine; road rows stream to DRAM and a
    single dma_gather packs the per-trajectory rows (host scatters).
All arithmetic on device; host does sharding, padding, and index layout.
"""

import os
import numpy as np
import ml_dtypes

import concourse.bass as bass
import concourse.bacc as bacc
import concourse.tile as tile
from concourse import mybir
from concourse.bass_utils import run_bass_kernel_spmd
from concourse.masks import make_identity

BF16 = ml_dtypes.bfloat16
P = 128
N, E, D = 100000, 1600000, 128
NCORES = 8
NB = 98                   # bins (blocks) per core
NBINS = NCORES * NB       # 784
NSPAD = NB * P            # 12544 road rows per core

SEGQ = [5, 4, 4, 4]       # chunk quota per segment
CB = sum(SEGQ)            # 17 chunks per block
SOFF = [0, 5, 9, 13]      # chunk offset of each segment within a block
SEGSZ = [32768, 28672, 28672, 28672]
SEGBASE = [0, 32768, 61440, 90112]
UFROWS = 118784
GB = 7                    # blocks per gather group
NGRP = NB // GB           # 14
J = NB * CB               # 1666 chunks per core
GCOLS = [GB * q * P // 16 for q in SEGQ]     # idx columns per (group, seg)
GSTRIDE = sum(GCOLS)                          # 952 idx columns per group

F32 = mybir.dt.float32
BF = mybir.dt.bfloat16
I32 = mybir.dt.int32
I16 = mybir.dt.int16

LAST_EXEC_NS = None
LAST_EXEC_PARTS = None


def _build_kernel_a(padw, gsz):
    ngrp = (NB + gsz - 1) // gsz
    assert ngrp * gsz == NB
    nc = bacc.Bacc("TRN2", target_bir_lowering=False,
                   debug=bool(os.environ.get("KERNEL_DEBUG")))
    wpad = nc.dram_tensor("wpad", [P, NB * padw], F32, kind="ExternalInput")
    nfs = nc.dram_tensor("nfs", [P, NB * P], F32, kind="ExternalInput")
    u_sh = nc.dram_tensor("u_sh", [P, NB * P], BF, kind="ExternalOutput")
    dinv_sh = nc.dram_tensor("dinv_sh", [P, NB], F32, kind="ExternalOutput")
    with tile.TileContext(nc) as tc:
        with tc.tile_pool(name="sb", bufs=1) as sb, \
             tc.tile_pool(name="nfp", bufs=3) as nfp, \
             tc.tile_pool(name="ubp", bufs=3) as ubp:
            w_sb = sb.tile([P, NB * padw], F32)
            nc.sync.dma_start(w_sb[:], wpad[:])
            deg = sb.tile([P, NB], F32)
            nc.vector.tensor_reduce(
                out=deg[:],
                in_=w_sb[:].rearrange("p (b s) -> p b s", s=padw),
                axis=mybir.AxisListType.X,
                op=mybir.AluOpType.add,
            )
            nc.vector.tensor_scalar_add(deg[:], deg[:], 1.0)  # self loop
            rec = sb.tile([P, NB], F32)
            nc.vector.reciprocal(rec[:], deg[:])
            dinv = sb.tile([P, NB], F32)
            nc.scalar.activation(dinv[:], rec[:], mybir.ActivationFunctionType.Sqrt)
            nc.sync.dma_start(dinv_sh[:], dinv[:])
            for g in range(ngrp):
                c0, c1 = g * gsz * P, (g + 1) * gsz * P
                t = nfp.tile([P, gsz * P], F32, tag="nf")
                nc.sync.dma_start(t[:], nfs[:, c0:c1])
                ub = ubp.tile([P, gsz * P], BF, tag="ub")
                nc.vector.tensor_tensor(
                    out=ub[:].rearrange("p (b f) -> p b f", b=gsz),
                    in0=t[:].rearrange("p (b f) -> p b f", b=gsz),
                    in1=dinv[:, g * gsz:(g + 1) * gsz].unsqueeze(2)
                        .broadcast_to([P, gsz, P]),
                    op=mybir.AluOpType.mult,
                )
                nc.sync.dma_start(u_sh[:, c0:c1], ub[:])
    nc.compile()
    return nc


def _build_kernel_b(j2, has_bias):
    nc = bacc.Bacc("TRN2", target_bir_lowering=False,
                   debug=bool(os.environ.get("KERNEL_DEBUG")))
    u_full = nc.dram_tensor("u_full", [UFROWS, P], BF, kind="ExternalInput")
    ulT = nc.dram_tensor("ulT", [P, NSPAD], BF, kind="ExternalInput")
    gidx = nc.dram_tensor("gidx", [P, NGRP * GSTRIDE], I16, kind="ExternalInput")
    cl = nc.dram_tensor("cl", [P, J], BF, kind="ExternalInput")
    wch = nc.dram_tensor("wch", [P, J], BF, kind="ExternalInput")
    dinvb = nc.dram_tensor("dinvb", [P, NB], F32, kind="ExternalInput")
    wt = nc.dram_tensor("wt", [P, P], BF, kind="ExternalInput")
    oidx = nc.dram_tensor("oidx", [P, j2 * 8], I16, kind="ExternalInput")
    out_packed = nc.dram_tensor("out_packed", [j2 * P, P], BF, kind="ExternalOutput")
    if has_bias:
        bb = nc.dram_tensor("bb", [P, P], F32, kind="ExternalInput")

    with tile.TileContext(nc) as tc:
        with tc.tile_pool(name="sb", bufs=1) as sb, \
             tc.tile_pool(name="gp", bufs=2) as gp, \
             tc.tile_pool(name="mp", bufs=2) as mp, \
             tc.tile_pool(name="blk", bufs=3) as blk, \
             tc.tile_pool(name="ps", bufs=3, space="PSUM") as ps, \
             tc.tile_pool(name="ps2", bufs=2, space="PSUM") as ps2, \
             tc.tile_pool(name="dram", bufs=1, space="DRAM") as dr:
            gidx_sb = sb.tile([P, NGRP * GSTRIDE], I16)
            nc.sync.dma_start(gidx_sb[:], gidx[:])
            cl_sb = sb.tile([P, J], BF)
            nc.sync.dma_start(cl_sb[:], cl[:])
            w_sb = sb.tile([P, J], BF)
            nc.sync.dma_start(w_sb[:], wch[:])
            dinv_sb = sb.tile([P, NB], F32)
            nc.sync.dma_start(dinv_sb[:], dinvb[:])
            wt_sb = sb.tile([P, P], BF)
            nc.sync.dma_start(wt_sb[:], wt[:])
            ulT_sb = sb.tile([P, NSPAD], BF)
            nc.sync.dma_start(ulT_sb[:], ulT[:])
            oidx_sb = sb.tile([P, j2 * 8], I16)
            nc.sync.dma_start(oidx_sb[:], oidx[:])
            if has_bias:
                bb_sb = sb.tile([P, P], F32)
                nc.sync.dma_start(bb_sb[:], bb[:])

            iota_i = sb.tile([P, P], I32)
            nc.gpsimd.iota(iota_i[:], pattern=[[1, P]], channel_multiplier=0)
            iota_f = sb.tile([P, P], F32)
            nc.vector.tensor_copy(iota_f[:], iota_i[:])
            iota_bf = sb.tile([P, P], BF)
            nc.vector.tensor_copy(iota_bf[:], iota_f[:])
            identf = sb.tile([P, P], F32)
            make_identity(nc, identf[:])
            ident = sb.tile([P, P], BF)
            nc.vector.tensor_copy(ident[:], identf[:])

            road = dr.tile([NSPAD, P], BF)
            cl3 = cl_sb[:].rearrange("p (b k) -> p b k", b=NB)
            w3 = w_sb[:].rearrange("p (b k) -> p b k", b=NB)

            for g in range(NGRP):
                ugs, Ms = [], []
                coff = g * GSTRIDE
                for s in range(4):
                    q = SEGQ[s]
                    nidx = GB * q * P
                    ug = gp.tile([P, GB * q * P], BF, tag=f"ug{s}")
                    nc.gpsimd.dma_gather(
                        ug[:].rearrange("p (c f) -> p c f", f=P),
                        u_full[SEGBASE[s]:SEGBASE[s] + SEGSZ[s], :],
                        gidx_sb[:, coff:coff + nidx // 16],
                        nidx, nidx, P, single_packet=False)
                    coff += nidx // 16
                    M = mp.tile([P, GB * q * P], BF, tag=f"m{s}")
                    m4 = M[:].rearrange("p (b k f) -> p b k f", b=GB, k=q)
                    csl = cl3[:, g * GB:(g + 1) * GB, SOFF[s]:SOFF[s] + q]
                    nc.vector.tensor_tensor(
                        out=m4,
                        in0=iota_bf[:].unsqueeze(1).unsqueeze(1)
                            .broadcast_to([P, GB, q, P]),
                        in1=csl.unsqueeze(3).broadcast_to([P, GB, q, P]),
                        op=mybir.AluOpType.is_equal,
                    )
                    wsl = w3[:, g * GB:(g + 1) * GB, SOFF[s]:SOFF[s] + q]
                    nc.vector.tensor_tensor(
                        out=m4, in0=m4,
                        in1=wsl.unsqueeze(3).broadcast_to([P, GB, q, P]),
                        op=mybir.AluOpType.mult,
                    )
                    ugs.append(ug)
                    Ms.append(M)

                for b_in in range(GB):
                    bi = g * GB + b_in
                    zpT = ps.tile([P, P], F32, tag="zpT")
                    nc.tensor.matmul(zpT[:], lhsT=ident[:],
                                     rhs=ulT_sb[:, bi * P:(bi + 1) * P],
                                     start=True, stop=False)
                    nmm = 0
                    for s in range(4):
                        q = SEGQ[s]
                        for c in range(q):
                            off = (b_in * q + c) * P
                            nmm += 1
                            nc.tensor.matmul(
                                zpT[:],
                                lhsT=ugs[s][:, off:off + P],
                                rhs=Ms[s][:, off:off + P],
                                start=False, stop=(nmm == CB))
                    zs = blk.tile([P, P], BF, tag="zs")
                    nc.scalar.activation(zs[:], zpT[:],
                                         mybir.ActivationFunctionType.Copy)
                    t2 = ps2.tile([P, P], F32, tag="t2")
                    nc.tensor.matmul(t2[:], lhsT=zs[:], rhs=wt_sb[:],
                                     start=True, stop=True)
                    r1 = blk.tile([P, P], BF, tag="r1")
                    if has_bias:
                        r0 = blk.tile([P, P], F32, tag="r0")
                        nc.vector.scalar_tensor_tensor(
                            out=r0[:], in0=t2[:],
                            scalar=dinv_sb[:, bi:bi + 1],
                            in1=bb_sb[:], op0=mybir.AluOpType.mult,
                            op1=mybir.AluOpType.add)
                        nc.scalar.activation(r1[:], r0[:],
                                             mybir.ActivationFunctionType.Relu)
                    else:
                        nc.scalar.activation(r1[:], t2[:],
                                             mybir.ActivationFunctionType.Relu,
                                             scale=dinv_sb[:, bi:bi + 1])
                    nc.sync.dma_start(road[bi * P:(bi + 1) * P, :], r1[:])

            og = sb.tile([P, j2 * P], BF)
            nc.gpsimd.dma_gather(
                og[:].rearrange("p (c f) -> p c f", f=P),
                road[:], oidx_sb[:, :], j2 * P, j2 * P, P,
                single_packet=False)
            nc.sync.dma_start(
                out_packed[:].rearrange("(j p) f -> p j f", p=P),
                og[:].rearrange("p (j f) -> p j f", j=j2))
    nc.compile()
    return nc


def _assign_segments(row, ebin):
    """Greedy per-node segment choice (desc out-degree) honoring per-cell
    chunk quotas and segment populations."""
    qcap = np.array([q * P for q in SEGQ], np.int64)
    popcap = np.array(SEGSZ, np.int64)
    eo = np.argsort(row, kind="stable")
    ebin_s = ebin[eo]
    outdeg = np.bincount(row, minlength=N)
    rstart = np.zeros(N + 1, np.int64)
    np.cumsum(outdeg, out=rstart[1:])
    nodes = np.argsort(-outdeg, kind="stable")
    cell = np.zeros((NBINS, 4), np.int64)
    pop = np.zeros(4, np.int64)
    seg = np.full(N, -1, np.int8)
    for n in nodes:
        bins = ebin_s[rstart[n]:rstart[n + 1]]
        if len(bins):
            bc_bins, bc_cnt = np.unique(bins, return_counts=True)
            proj = cell[bc_bins, :] + bc_cnt[:, None]
            overflow = np.maximum(proj - qcap[None, :], 0).sum(0)
            ratio = (proj.astype(np.float64) / qcap[None, :]).max(0)
            score = overflow * 1000 + ratio
        else:
            score = np.zeros(4)
        score = np.where(pop >= popcap, np.inf, score + 1e-6 * (pop / popcap))
        s = int(np.argmin(score))
        seg[n] = s
        pop[s] += 1
        if len(bins):
            np.add.at(cell, (bc_bins, np.full(len(bc_bins), s)), bc_cnt)
    assert (cell <= qcap[None, :]).all(), "segment quota violated"
    return seg


def _wrap16(stream):
    """dma_gather index layout: position i -> idxs[16*g + i%16, i//16],
    replicated across the 8 Q7 core groups."""
    w16 = stream.reshape(-1, 16).T
    return np.ascontiguousarray(np.tile(w16, (8, 1)))


def kernel(**inputs):
    traj = np.asarray(inputs["traj_seqs"])[..., 0].astype(np.int64)
    seq_len = np.asarray(inputs["seq_len"]).astype(np.int64)
    nf = np.ascontiguousarray(np.asarray(inputs["node_feat"], dtype=np.float32))
    ei = np.asarray(inputs["edge_index"]).astype(np.int64)
    ef = np.asarray(inputs["edge_feat"], dtype=np.float32)
    W = np.ascontiguousarray(np.asarray(inputs["W"], dtype=np.float32))
    b = np.asarray(inputs["b"], dtype=np.float32)

    row, col = ei[0], ei[1]

    # ---------- host layout prep ----------
    cnt = np.bincount(col, minlength=N).astype(np.int64)
    order = np.argsort(-cnt, kind="stable")
    nodebin = np.empty(N, np.int32)
    nodeslot = np.empty(N, np.int32)
    fwd = np.arange(NBINS, dtype=np.int32)
    for r in range((N + NBINS - 1) // NBINS):
        idx = order[r * NBINS:(r + 1) * NBINS]
        bins = fwd[:len(idx)] if r % 2 == 0 else fwd[::-1][:len(idx)]
        nodebin[idx] = bins
        nodeslot[idx] = r
    ncore = nodebin // NB
    nblk = nodebin % NB

    ebin = nodebin[col]
    seg = _assign_segments(row, ebin)

    # u_full positions: nodes of each segment packed by id order
    upos = np.empty(N, np.int64)
    for s in range(4):
        ns = np.where(seg == s)[0]
        assert len(ns) <= SEGSZ[s]
        upos[ns] = SEGBASE[s] + np.arange(len(ns))

    # edge -> (core, chunk_col, slot)
    eseg = seg[row].astype(np.int64)
    ecell = ebin * 4 + eseg
    cellcnt = np.bincount(ecell, minlength=NBINS * 4)
    eorder = np.argsort(ecell, kind="stable")
    cstarts = np.zeros(NBINS * 4, np.int64)
    np.cumsum(cellcnt[:-1], out=cstarts[1:])
    pos = np.arange(E, dtype=np.int64) - cstarts[ecell[eorder]]
    so = np.asarray(SOFF, np.int64)
    e_core = (ebin[eorder] // NB)
    e_blk = (ebin[eorder] % NB)
    e_chunk = e_blk * CB + so[eseg[eorder]] + pos // P
    e_p = pos % P

    idx_a = np.zeros((NCORES, P, J), np.int16)
    cl_a = np.zeros((NCORES, P, J), BF16)
    w_a = np.zeros((NCORES, P, J), BF16)
    segbase_arr = np.asarray(SEGBASE, np.int64)
    idx_a[e_core, e_p, e_chunk] = (upos[row[eorder]]
                                   - segbase_arr[eseg[eorder]]).astype(np.int16)
    cl_a[e_core, e_p, e_chunk] = nodeslot[col[eorder]].astype(np.float32)
    w_a[e_core, e_p, e_chunk] = ef[eorder]

    # gather stream column order: (group, seg, block-in-group, chunk)
    allcols = np.concatenate([
        np.array([b * CB + SOFF[s] + c
                  for s in range(4)
                  for b in range(g * GB, (g + 1) * GB)
                  for c in range(SEGQ[s])], np.int64)
        for g in range(NGRP)])
    assert len(allcols) == J

    # per-node weight table for deg (padw = max in-degree)
    padw = int(cnt.max())
    norder = np.argsort(col, kind="stable")
    ncol = col[norder]
    nstarts = np.zeros(N, np.int64)
    np.cumsum(cnt[:-1], out=nstarts[1:])
    npos = np.arange(E, dtype=np.int64) - nstarts[ncol]
    wpad_a = np.zeros((NCORES, P, NB * padw), np.float32)
    wpad_a[ncore[ncol], nodeslot[ncol], nblk[ncol] * padw + npos] = ef[norder]

    nfs_a = np.zeros((NCORES, P, NB, P), np.float32)
    nfs_a[ncore, nodeslot, nblk] = nf
    nfs_a = nfs_a.reshape(NCORES, P, NB * P)

    # trajectory gather prep
    flat = traj.reshape(-1)
    posmask = (np.arange(512)[None, :] < seq_len[:, None]).reshape(-1)
    roadrow = nblk.astype(np.int64) * P + nodeslot.astype(np.int64)
    oo = ncore[flat]
    sels = [np.where((oo == k) & posmask)[0] for k in range(NCORES)]
    j2 = max(1, int(np.ceil(max(len(s) for s in sels) / P)))

    trace = bool(os.environ.get("KERNEL_TRACE"))

    # ---------- launch A ----------
    in_maps_a = [{"wpad": wpad_a[k], "nfs": nfs_a[k]} for k in range(NCORES)]
    nca = _build_kernel_a(padw, gsz=14)
    ra = run_bass_kernel_spmd(nca, in_maps_a, core_ids=list(range(NCORES)),
                              trace=trace)

    u_full = np.zeros((UFROWS, P), BF16)
    for k in range(NCORES):
        ush = ra.results[k]["u_sh"].reshape(P, NB, P)  # [slot, blk, feat]
        m = ncore == k
        u_full[upos[m]] = ush[nodeslot[m], nblk[m]]
    u_full = np.ascontiguousarray(u_full)

    # ---------- launch B ----------
    has_bias = bool(np.any(b != 0.0))
    in_maps_b = []
    for k in range(NCORES):
        ush = ra.results[k]["u_sh"].reshape(P, NB, P)
        ulT = np.ascontiguousarray(ush.transpose(2, 1, 0).reshape(P, NSPAD))
        gstream = idx_a[k][:, allcols].T.reshape(-1)
        ostream = np.zeros(j2 * P, np.int16)
        lv = roadrow[flat[sels[k]]].astype(np.int16)
        ostream[:len(lv)] = lv
        im = {
            "u_full": u_full, "ulT": ulT, "gidx": _wrap16(gstream),
            "cl": cl_a[k], "wch": w_a[k],
            "dinvb": ra.results[k]["dinv_sh"], "wt": W.astype(BF16),
            "oidx": _wrap16(ostream),
        }
        if has_bias:
            im["bb"] = np.ascontiguousarray(
                np.broadcast_to(b, (P, P))).astype(np.float32)
        in_maps_b.append(im)

    ncb = _build_kernel_b(j2, has_bias)
    rb = run_bass_kernel_spmd(ncb, in_maps_b, core_ids=list(range(NCORES)),
                              trace=trace)
    global LAST_EXEC_NS, LAST_EXEC_PARTS
    LAST_EXEC_PARTS = (ra.exec_time_ns, rb.exec_time_ns)
    if ra.exec_time_ns and rb.exec_time_ns:
        LAST_EXEC_NS = ra.exec_time_ns + rb.exec_time_ns

    # ---------- host assembly ----------
    out = np.zeros((64 * 512, D), np.float32)
    for k in range(NCORES):
        if len(sels[k]):
            out[sels[k]] = rb.results[k]["out_packed"][:len(sels[k])].astype(np.float32)
    return out.reshape(64, 512, D)
